# revision 4
# baseline (speedup 1.0000x reference)
"""Trainium2 Bass kernel for a 2-layer masked (ragged) Elman RNN — v3.

v2 structure (all-fp16 datapath, quadrant-packed PSUM, one wide ACT, fp16 PE
transposes, capture-from-hstack) plus:
  - phase A (embed + xp0) is interleaved INTO the layer-0 recurrence: one
    A-chunk (8 timesteps of xp0) is produced per 8 recurrence steps, handing
    xp to the recurrence through an SBUF ot-tile ring (no DRAM bounce).
  - phase C (xp1) is likewise interleaved into the layer-1 recurrence.
  - bulk GEMMs run k-outer/j-inner so a stationary tile is reused by 4
    consecutive matmuls.
  - per-step xp injection reads a 32-row aligned window of the ot tile and
    uses an identity-slice selector as the stationary (odd steps start at
    partition 16, which engines cannot address directly).

Weight residency per scope: {W_ih0 32KB, W_hh0 64KB} then {W_ih1 64KB,
W_hh1 64KB} per partition — both fit alongside ~40KB of working tiles.
"""

import sys

sys.path.insert(0, "/opt/trn_rl_repo")

import numpy as np

B, T, V, D, H = 128, 512, 32000, 1024, 2048
NC = 8
BL = B // NC          # 16 sequences per core
KT = H // 128         # 16 k-tiles of the hidden dim
DKT = D // 128        # 8 k-tiles of the embedding dim
NQ = 4                # 4 psum quadrants / n-blocks of 512

STATS = {}
_CACHE = {}


def _build(t_steps, debug=False):
    import concourse.bass as bass
    import concourse.mybir as mybir
    import concourse.tile as tile
    from concourse import bacc
    from concourse.masks import make_identity

    f32 = mybir.dt.float32
    f16 = mybir.dt.float16
    i32 = mybir.dt.int32
    Tanh = mybir.ActivationFunctionType.Tanh

    mt = (t_steps * BL) // 128   # 128-row token tiles == 8-step chunks
    nchunk = t_steps // 8
    assert mt == nchunk

    nc = bacc.Bacc("TRN2", target_bir_lowering=False, debug=False, num_devices=NC)

    tokT = nc.dram_tensor("tokT", [128, mt], i32, kind="ExternalInput")
    cap_idx = nc.dram_tensor("cap_idx", [128, 1], i32, kind="ExternalInput")
    emb = nc.dram_tensor("emb", [V, D], f16, kind="ExternalInput")
    w_ih0 = nc.dram_tensor("w_ih0", [D, H], f16, kind="ExternalInput")
    w_hh0 = nc.dram_tensor("w_hh0", [H, H], f16, kind="ExternalInput")
    b0 = nc.dram_tensor("b0", [1, H], f32, kind="ExternalInput")
    w_ih1 = nc.dram_tensor("w_ih1", [H, H], f16, kind="ExternalInput")
    w_hh1 = nc.dram_tensor("w_hh1", [H, H], f16, kind="ExternalInput")
    b1 = nc.dram_tensor("b1", [1, H], f32, kind="ExternalInput")
    out_h = nc.dram_tensor("out_h", [BL, H], f32, kind="ExternalOutput")

    kd = dict(kind="ExternalOutput") if debug else {}
    # y0T plane layout: [t, p, m*128 + 32j + b] = h0_t[b, (4j+m)*128+p]
    # (cols 32j+16..32j+31 within each m-block are don't-care)
    y0T_d = nc.dram_tensor("y0T_d", [t_steps, 128, 512], f16, **kd)
    # layer-1 tanh output in hstack layout: row t*128 + 32j + b = h[b, j*512:...]
    h1s_d = nc.dram_tensor("h1s_d", [t_steps * 128, 512], f16, **kd)

    def load_w(W_sb, wsrc, ktiles):
        for k in range(ktiles):
            nc.gpsimd.dma_start(
                W_sb[:, k * H:(k + 1) * H], wsrc[k * 128:(k + 1) * 128, :])

    def load_bias(bias_sb, bsrc):
        nc.gpsimd.dma_start(bias_sb[0:1, :], bsrc[0:1, :])
        nc.gpsimd.partition_broadcast(bias_sb[:], bias_sb[0:1, :])

    with tile.TileContext(nc) as tc:
        with tc.tile_pool(name="state", bufs=1) as st:
            ident = st.tile([128, 128], f16)
            make_identity(nc, ident[:])
            bias_sb = st.tile([128, H], f32)
            zero_sb = st.tile([128, KT * BL], f16)
            nc.gpsimd.memset(zero_sb[:], 0.0)
            tokens_sb = st.tile([128, mt], i32)
            nc.gpsimd.dma_start(tokens_sb[:], tokT[:, :])

            # ---- fused recurrence + bulk-xp producer --------------------
            # prefetch(c): issue the chunk-c input DMAs (2 chunks ahead)
            # produce(c): emit the chunk-c GEMM, returns [ot_j tiles] whose
            #             rows are t_loc*16+b covering steps 8c..8c+7
            def recurrence(layer, W_sb, prefetch, produce):
                # hT is kept as FOUR [128, 128] tiles hTm[m] — the raw XBAR
                # transpose of hstack col-block m. k-tile κ's stationary is
                # hTm[κ%4][:, 32*(κ//4) : +16]; cols 32j+16..32j+31 are
                # don't-care.
                with (
                    nc.named_scope(f"rec{layer}"),
                    tc.tile_pool(name=f"st{layer}", bufs=2) as stp,
                    tc.tile_pool(name=f"hs{layer}", bufs=2) as hsp,
                    tc.tile_pool(name=f"zt{layer}", bufs=2, space="PSUM") as ztp,
                ):
                    ring = {}
                    prefetch(0)
                    if nchunk > 1:
                        prefetch(1)
                    sl0, fin0 = produce(0)
                    for s in sl0:
                        s()
                    ring[0] = fin0()
                    cur_slices, cur_fin, cur_c = None, None, None

                    def new_hT():
                        out = []
                        for m in range(4):
                            hTm = stp.tile([128, 128], f16, tag=f"hT{m}",
                                           name=f"hT{m}")
                            out.append(hTm)
                        return out

                    hT = new_hT()
                    for m in range(4):
                        nc.vector.tensor_copy(hT[m][:], zero_sb[:, 0:128])
                    for t in range(t_steps):
                        c, r = t // 8, t % 8
                        if r == 0:
                            if c + 2 < nchunk:
                                prefetch(c + 2)
                            if c + 1 < nchunk:
                                cur_slices, cur_fin, cur_c = *produce(c + 1), c + 1
                            else:
                                cur_slices, cur_fin, cur_c = None, None, None
                            ring.pop(c - 1, None)
                        ots = ring[c]
                        w, half = (r // 2) * 32, r % 2
                        zt = ztp.tile([128, 512], f32, space="PSUM")
                        for j in range(NQ):
                            # zt[32j+b, :] = ot_j[r*16+b, :] via an identity
                            # selector on a 32-aligned window
                            nc.tensor.matmul(
                                zt[32 * j:32 * j + BL, :],
                                lhsT=ident[w:w + 32,
                                           w + half * BL:w + half * BL + BL],
                                rhs=ots[j][w:w + 32, :],
                                start=True, stop=False,
                                tile_position=(w, 32 * j),
                            )
                        for m in range(4):
                            for k in (m, m + 4, m + 8, m + 12):
                                for j in range(NQ):
                                    nc.tensor.matmul(
                                        zt[32 * j:32 * j + BL, :],
                                        lhsT=hT[k % 4][:, 32 * (k // 4):
                                                       32 * (k // 4) + BL],
                                        rhs=W_sb[:, k * H + j * 512:
                                                 k * H + (j + 1) * 512],
                                        start=False, stop=(k == KT - 1),
                                        tile_position=(0, 32 * j),
                                    )
                        # bulk-GEMM slice fills the PE while ACT+transposes run
                        if cur_slices is not None:
                            cur_slices[r]()
                            if r == 7:
                                ring[cur_c] = cur_fin()
                        hs = hsp.tile([128, 512], f16)
                        nc.scalar.activation(hs[:], zt[:], Tanh)
                        if layer == 1:
                            nc.gpsimd.dma_start(
                                h1s_d[t * 128:(t + 1) * 128, :], hs[:])
                        hT = new_hT()
                        for m in range(4):
                            nc.sync.dma_start_transpose(
                                hT[m][:], hs[:, m * 128:(m + 1) * 128])
                            if layer == 0:
                                nc.gpsimd.dma_start(
                                    y0T_d[t, :, m * 128:(m + 1) * 128],
                                    hT[m][:])

            # ================= scope 1: phase A + rec0 ===================
            load_bias(bias_sb, b0)
            with (
                tc.tile_pool(name="wih0", bufs=1) as wap,
                tc.tile_pool(name="whh0", bufs=1) as wp,
                tc.tile_pool(name="ga", bufs=3) as gp,
                tc.tile_pool(name="xt", bufs=2) as xtp,
                tc.tile_pool(name="pa", bufs=1, space="PSUM") as pap,
                tc.tile_pool(name="ota", bufs=12) as otp,
            ):
                WA_sb = wap.tile([128, DKT * H], f16)     # 32KB/partition
                load_w(WA_sb, w_ih0, DKT)
                W_sb = wp.tile([128, KT * H], f16)        # 64KB/partition
                load_w(W_sb, w_hh0, KT)

                xg_ring = {}

                def prefetchA(c):
                    xg = gp.tile([128, D], f16)
                    nc.gpsimd.indirect_dma_start(
                        out=xg[:], out_offset=None,
                        in_=emb[:],
                        in_offset=bass.IndirectOffsetOnAxis(
                            ap=tokens_sb[:, c:c + 1], axis=0),
                    )
                    xg_ring[c] = xg

                def produceA(c):
                    xg = xg_ring.pop(c)
                    xts = []
                    for k in range(DKT):
                        xtm = xtp.tile([128, 128], f16, name=f"xt{k}")
                        nc.sync.dma_start_transpose(
                            xtm[:], xg[:, k * 128:(k + 1) * 128])
                        xts.append(xtm)
                    pss = []
                    for j in range(NQ):
                        psa = pap.tile([128, 512], f32, space="PSUM",
                                       name=f"psa{j}")
                        pss.append(psa)

                    def mk(k):
                        def slice_fn():
                            for j in range(NQ):
                                nc.tensor.matmul(
                                    pss[j][:],
                                    lhsT=xts[k][:],
                                    rhs=WA_sb[:, k * H + j * 512:
                                              k * H + (j + 1) * 512],
                                    start=(k == 0), stop=(k == DKT - 1),
                                )
                        return slice_fn

                    def finish():
                        ots = []
                        for j in range(NQ):
                            ot = otp.tile([128, 512], f16)
                            nc.vector.tensor_add(
                                ot[:], pss[j][:],
                                bias_sb[:, j * 512:(j + 1) * 512])
                            ots.append(ot)
                        return ots

                    return [mk(k) for k in range(DKT)], finish

                recurrence(0, W_sb, prefetchA, produceA)

            # ================= scope 2: phase C + rec1 ===================
            load_bias(bias_sb, b1)
            with (
                tc.tile_pool(name="wih1", bufs=1) as wap,
                tc.tile_pool(name="whh1", bufs=1) as wp,
                tc.tile_pool(name="lh", bufs=2) as lhp,
                tc.tile_pool(name="pc", bufs=1, space="PSUM") as pcp,
                tc.tile_pool(name="otc", bufs=12) as otp,
            ):
                WA_sb = wap.tile([128, KT * H], f16)      # 64KB/partition
                load_w(WA_sb, w_ih1, KT)
                W_sb = wp.tile([128, KT * H], f16)        # 64KB/partition
                load_w(W_sb, w_hh1, KT)

                lh_ring = {}

                def prefetchC(c):
                    lh0 = lhp.tile([128, 8 * 512], f16)
                    nc.gpsimd.dma_start(
                        lh0[:].rearrange("p (t c) -> p t c", t=8),
                        y0T_d[c * 8:(c + 1) * 8, :, :].rearrange("t p c -> p t c"),
                    )
                    lh_ring[c] = lh0

                def produceC(c):
                    lh0 = lh_ring.pop(c)
                    # y0T col t*512 + m*128 + 32j + b (b<16 valid) ->
                    # lh col (4j+m)*128 + t*16 + b so each k-tile's 128
                    # stationary columns are contiguous
                    lh = lhp.tile([128, 8 * KT * BL], f16)
                    nc.vector.tensor_copy(
                        lh[:].rearrange("p (j m2 t b) -> p j m2 t b",
                                        j=4, m2=4, t=8),
                        lh0[:].rearrange("p (t m2 j2 b2) -> p j2 m2 t b2",
                                         t=8, m2=4, j2=4)[:, :, :, :, 0:BL],
                    )
                    pss = []
                    for j in range(NQ):
                        psc = pcp.tile([128, 512], f32, space="PSUM",
                                       name=f"psc{j}")
                        pss.append(psc)

                    def mk(s):
                        def slice_fn():
                            for k in (2 * s, 2 * s + 1):
                                for j in range(NQ):
                                    nc.tensor.matmul(
                                        pss[j][:],
                                        lhsT=lh[:, k * 128:(k + 1) * 128],
                                        rhs=WA_sb[:, k * H + j * 512:
                                                  k * H + (j + 1) * 512],
                                        start=(k == 0), stop=(k == KT - 1),
                                    )
                        return slice_fn

                    def finish():
                        ots = []
                        for j in range(NQ):
                            ot = otp.tile([128, 512], f16)
                            nc.vector.tensor_add(
                                ot[:], pss[j][:],
                                bias_sb[:, j * 512:(j + 1) * 512])
                            ots.append(ot)
                        return ots

                    return [mk(s) for s in range(8)], finish

                recurrence(1, W_sb, prefetchC, produceC)

            # final capture: out[b] = h1 (fp16, already tanh'd) at t = len_b-1
            with tc.tile_pool(name="cap", bufs=1) as cp:
                ci = cp.tile([128, 1], i32)
                nc.gpsimd.dma_start(ci[:], cap_idx[:, :])
                og = cp.tile([128, 512], f16)
                nc.gpsimd.indirect_dma_start(
                    out=og[:], out_offset=None,
                    in_=h1s_d[:],
                    in_offset=bass.IndirectOffsetOnAxis(ap=ci[:, :1], axis=0),
                )
                oh = cp.tile([128, 512], f32)
                nc.vector.tensor_copy(oh[:], og[:])
                for j in range(NQ):
                    nc.gpsimd.dma_start(
                        out_h[:, j * 512:(j + 1) * 512],
                        oh[32 * j:32 * j + BL, :])

    nc.finalize()
    return nc


def _install_ntff_hook():
    """The trimmed agent image lacks antenv.axon_hooks — provide the tiny
    get/set registry and install the ctypes NTFF hook so trace=True works."""
    import types

    if "antenv.axon_hooks" in sys.modules:
        return
    m = types.ModuleType("antenv.axon_hooks")
    _hook = [None]
    m.set_axon_ntff_profile_hook = lambda h: _hook.__setitem__(0, h)
    m.get_axon_ntff_profile_hook = lambda: _hook[0]
    sys.modules["antenv.axon_hooks"] = m
    import antenv
    antenv.axon_hooks = m
    try:
        from trn_agent_boot.trn_boot import _ntff_profile_via_ctypes
        hook = _ntff_profile_via_ctypes("/opt/axon/libaxon_pjrt.so")
        if hook is not None:
            m.set_axon_ntff_profile_hook(hook)
        import concourse.bass_utils as bu
        bu.upload_artifacts = lambda d: str(d)
    except Exception:
        pass


def kernel(tokens, lengths, emb, W_ih0, W_hh0, b0, W_ih1, W_hh1, b1,
           _t_steps=T, _trace=False, _debug=False):
    from concourse.bass_utils import run_bass_kernel_spmd

    if _trace:
        _install_ntff_hook()

    tokens = np.asarray(tokens).astype(np.int32)
    lengths = np.asarray(lengths).astype(np.int32)
    emb16 = np.ascontiguousarray(np.asarray(emb, dtype=np.float16))
    W_ih0 = np.ascontiguousarray(np.asarray(W_ih0, dtype=np.float16))
    W_hh0 = np.ascontiguousarray(np.asarray(W_hh0, dtype=np.float16))
    W_ih1 = np.ascontiguousarray(np.asarray(W_ih1, dtype=np.float16))
    W_hh1 = np.ascontiguousarray(np.asarray(W_hh1, dtype=np.float16))
    b0 = np.ascontiguousarray(np.asarray(b0, dtype=np.float32).reshape(1, H))
    b1 = np.ascontiguousarray(np.asarray(b1, dtype=np.float32).reshape(1, H))

    ts = _t_steps
    key = (ts, _debug)
    if key not in _CACHE:
        _CACHE[key] = _build(ts, _debug)
    nc = _CACHE[key]

    in_maps = []
    for c in range(NC):
        tok_c = tokens[c * BL:(c + 1) * BL, :ts]          # [16, ts]
        flat = tok_c.T.reshape(-1)                        # t-major rows
        tokTc = np.ascontiguousarray(flat.reshape(-1, 128).T)  # [128, mt]
        len_c = np.minimum(lengths[c * BL:(c + 1) * BL].astype(np.int64), ts)
        # capture row for og row 32j+b: (len_b-1)*128 + 32j + b (hstack
        # layout); rows 32j+16..32j+31 are dummies pointing at row 0
        cap = np.zeros((128, 1), np.int32)
        for j in range(4):
            cap[32 * j:32 * j + BL, 0] = (len_c - 1) * 128 + 32 * j + np.arange(BL)
        in_maps.append({
            "tokT": tokTc,
            "cap_idx": np.ascontiguousarray(cap),
            "emb": emb16,
            "w_ih0": W_ih0, "w_hh0": W_hh0, "b0": b0,
            "w_ih1": W_ih1, "w_hh1": W_hh1, "b1": b1,
        })

    res = run_bass_kernel_spmd(nc, in_maps, list(range(NC)), trace=_trace)
    STATS["exec_time_ns"] = res.exec_time_ns
    STATS["mean_exec_time_ns"] = res.mean_exec_time_ns
    STATS["scope_times"] = res.per_core_scope_times
    if _debug:
        STATS["debug"] = res.results
    out = np.concatenate([res.results[c]["out_h"] for c in range(NC)], axis=0)
    return out.astype(np.float32)


# revision 5
# speedup vs baseline: 1.5889x; 1.5889x over previous
"""Trainium2 Bass kernel for a 2-layer masked (ragged) Elman RNN — v3.

v2 structure (all-fp16 datapath, quadrant-packed PSUM, one wide ACT, fp16 PE
transposes, capture-from-hstack) plus:
  - phase A (embed + xp0) is interleaved INTO the layer-0 recurrence: one
    A-chunk (8 timesteps of xp0) is produced per 8 recurrence steps, handing
    xp to the recurrence through an SBUF ot-tile ring (no DRAM bounce).
  - phase C (xp1) is likewise interleaved into the layer-1 recurrence.
  - bulk GEMMs run k-outer/j-inner so a stationary tile is reused by 4
    consecutive matmuls.
  - per-step xp injection reads a 32-row aligned window of the ot tile and
    uses an identity-slice selector as the stationary (odd steps start at
    partition 16, which engines cannot address directly).

Weight residency per scope: {W_ih0 32KB, W_hh0 64KB} then {W_ih1 64KB,
W_hh1 64KB} per partition — both fit alongside ~40KB of working tiles.
"""

import sys

sys.path.insert(0, "/opt/trn_rl_repo")

import numpy as np

B, T, V, D, H = 128, 512, 32000, 1024, 2048
NC = 8
BL = B // NC          # 16 sequences per core
KT = H // 128         # 16 k-tiles of the hidden dim
DKT = D // 128        # 8 k-tiles of the embedding dim
NQ = 4                # 4 psum quadrants / n-blocks of 512

STATS = {}
_CACHE = {}


def _build(t_steps, debug=False):
    import concourse.bass as bass
    import concourse.mybir as mybir
    import concourse.tile as tile
    from concourse import bacc
    from concourse.masks import make_identity

    f32 = mybir.dt.float32
    f16 = mybir.dt.float16
    i32 = mybir.dt.int32
    Tanh = mybir.ActivationFunctionType.Tanh

    mt = (t_steps * BL) // 128   # 128-row token tiles == 8-step chunks
    nchunk = t_steps // 8
    assert mt == nchunk

    nc = bacc.Bacc("TRN2", target_bir_lowering=False, debug=False, num_devices=NC)

    tokT = nc.dram_tensor("tokT", [128, mt], i32, kind="ExternalInput")
    cap_idx = nc.dram_tensor("cap_idx", [128, 1], i32, kind="ExternalInput")
    emb = nc.dram_tensor("emb", [V, D], f16, kind="ExternalInput")
    w_ih0 = nc.dram_tensor("w_ih0", [D, H], f16, kind="ExternalInput")
    w_hh0 = nc.dram_tensor("w_hh0", [H, H], f16, kind="ExternalInput")
    b0 = nc.dram_tensor("b0", [1, H], f32, kind="ExternalInput")
    w_ih1 = nc.dram_tensor("w_ih1", [H, H], f16, kind="ExternalInput")
    w_hh1 = nc.dram_tensor("w_hh1", [H, H], f16, kind="ExternalInput")
    b1 = nc.dram_tensor("b1", [1, H], f32, kind="ExternalInput")
    out_h = nc.dram_tensor("out_h", [BL, H], f32, kind="ExternalOutput")

    kd = dict(kind="ExternalOutput") if debug else {}
    # y0T plane layout: [t, p, m*128 + 32j + b] = h0_t[b, (4j+m)*128+p]
    # (cols 32j+16..32j+31 within each m-block are don't-care)
    y0T_d = nc.dram_tensor("y0T_d", [t_steps, 128, 512], f16, **kd)
    # layer-1 tanh output in hstack layout: row t*128 + 32j + b = h[b, j*512:...]
    h1s_d = nc.dram_tensor("h1s_d", [t_steps * 128, 512], f16, **kd)

    def load_w(W_sb, wsrc, ktiles):
        for k in range(ktiles):
            nc.gpsimd.dma_start(
                W_sb[:, k * H:(k + 1) * H], wsrc[k * 128:(k + 1) * 128, :])

    def load_bias(bias_sb, bsrc):
        nc.gpsimd.dma_start(bias_sb[0:1, :], bsrc[0:1, :])
        nc.gpsimd.partition_broadcast(bias_sb[:], bias_sb[0:1, :])

    with tile.TileContext(nc) as tc:
        with tc.tile_pool(name="state", bufs=1) as st:
            ident = st.tile([128, 128], f16)
            make_identity(nc, ident[:])
            bias_sb = st.tile([128, H], f32)
            zero_sb = st.tile([128, KT * BL], f16)
            nc.gpsimd.memset(zero_sb[:], 0.0)
            tokens_sb = st.tile([128, mt], i32)
            nc.gpsimd.dma_start(tokens_sb[:], tokT[:, :])

            # ---- fused recurrence + bulk-xp producer --------------------
            # prefetch(c): issue the chunk-c input DMAs (2 chunks ahead)
            # produce(c): emit the chunk-c GEMM, returns [ot_j tiles] whose
            #             rows are t_loc*16+b covering steps 8c..8c+7
            def recurrence(layer, W_sb, prefetch, produce):
                # hT is kept as FOUR [128, 128] tiles hTm[m] — the raw XBAR
                # transpose of hstack col-block m. k-tile κ's stationary is
                # hTm[κ%4][:, 32*(κ//4) : +16]; cols 32j+16..32j+31 are
                # don't-care.
                with (
                    nc.named_scope(f"rec{layer}"),
                    tc.tile_pool(name=f"st{layer}", bufs=2) as stp,
                    tc.tile_pool(name=f"hs{layer}", bufs=2) as hsp,
                    tc.tile_pool(name=f"zt{layer}", bufs=2, space="PSUM") as ztp,
                    tc.tile_pool(name=f"tb{layer}", bufs=1, space="PSUM") as tbp,
                ):
                    ring = {}
                    prefetch(0)
                    if nchunk > 1:
                        prefetch(1)
                    sl0, fin0 = produce(0)
                    for s in sl0:
                        s()
                    ring[0] = fin0()
                    cur_slices, cur_fin, cur_c = None, None, None

                    def new_hT():
                        out = []
                        for m in range(4):
                            hTm = stp.tile([128, 128], f16, tag=f"hT{m}",
                                           name=f"hT{m}")
                            out.append(hTm)
                        return out

                    hT = new_hT()
                    for m in range(4):
                        nc.vector.tensor_copy(hT[m][:], zero_sb[:, 0:128])
                    for t in range(t_steps):
                        c, r = t // 8, t % 8
                        if r == 0:
                            if c + 2 < nchunk:
                                prefetch(c + 2)
                            if c + 1 < nchunk:
                                cur_slices, cur_fin, cur_c = *produce(c + 1), c + 1
                            else:
                                cur_slices, cur_fin, cur_c = None, None, None
                            ring.pop(c - 1, None)
                        ots = ring[c]
                        w, half = (r // 2) * 32, r % 2
                        zt = ztp.tile([128, 512], f32, space="PSUM")
                        for j in range(NQ):
                            # zt[32j+b, :] = ot_j[r*16+b, :] via an identity
                            # selector on a 32-aligned window
                            nc.tensor.matmul(
                                zt[32 * j:32 * j + BL, :],
                                lhsT=ident[w:w + 32,
                                           w + half * BL:w + half * BL + BL],
                                rhs=ots[j][w:w + 32, :],
                                start=True, stop=False,
                                tile_position=(w, 32 * j),
                            )
                        for m in range(4):
                            for k in (m, m + 4, m + 8, m + 12):
                                for j in range(NQ):
                                    nc.tensor.matmul(
                                        zt[32 * j:32 * j + BL, :],
                                        lhsT=hT[k % 4][:, 32 * (k // 4):
                                                       32 * (k // 4) + BL],
                                        rhs=W_sb[:, k * H + j * 512:
                                                 k * H + (j + 1) * 512],
                                        start=False, stop=(k == KT - 1),
                                        tile_position=(0, 32 * j),
                                    )
                        # bulk-GEMM slice fills the PE while ACT+transposes run
                        if cur_slices is not None:
                            cur_slices[r]()
                            if r == 7:
                                ring[cur_c] = cur_fin()
                        hs = hsp.tile([128, 512], f16)
                        nc.scalar.activation(hs[:], zt[:], Tanh)
                        if layer == 1:
                            nc.gpsimd.dma_start(
                                h1s_d[t * 128:(t + 1) * 128, :], hs[:])
                        hT = new_hT()
                        tb = tbp.tile([128, 512], f16, space="PSUM")
                        for m in range(4):
                            nc.tensor.transpose(
                                tb[:, m * 128:(m + 1) * 128],
                                hs[:, m * 128:(m + 1) * 128],
                                ident[:],
                            )
                            nc.vector.tensor_copy(
                                hT[m][:], tb[:, m * 128:(m + 1) * 128])
                            if layer == 0:
                                nc.gpsimd.dma_start(
                                    y0T_d[t, :, m * 128:(m + 1) * 128],
                                    hT[m][:])

            # ================= scope 1: phase A + rec0 ===================
            load_bias(bias_sb, b0)
            with (
                tc.tile_pool(name="wih0", bufs=1) as wap,
                tc.tile_pool(name="whh0", bufs=1) as wp,
                tc.tile_pool(name="ga", bufs=3) as gp,
                tc.tile_pool(name="xt", bufs=2) as xtp,
                tc.tile_pool(name="pa", bufs=1, space="PSUM") as pap,
                tc.tile_pool(name="ota", bufs=12) as otp,
            ):
                WA_sb = wap.tile([128, DKT * H], f16)     # 32KB/partition
                load_w(WA_sb, w_ih0, DKT)
                W_sb = wp.tile([128, KT * H], f16)        # 64KB/partition
                load_w(W_sb, w_hh0, KT)

                xg_ring = {}

                def prefetchA(c):
                    xg = gp.tile([128, D], f16)
                    nc.gpsimd.indirect_dma_start(
                        out=xg[:], out_offset=None,
                        in_=emb[:],
                        in_offset=bass.IndirectOffsetOnAxis(
                            ap=tokens_sb[:, c:c + 1], axis=0),
                    )
                    xg_ring[c] = xg

                def produceA(c):
                    xg = xg_ring.pop(c)
                    xts = []
                    for k in range(DKT):
                        xtm = xtp.tile([128, 128], f16, name=f"xt{k}")
                        nc.sync.dma_start_transpose(
                            xtm[:], xg[:, k * 128:(k + 1) * 128])
                        xts.append(xtm)
                    pss = []
                    for j in range(NQ):
                        psa = pap.tile([128, 512], f32, space="PSUM",
                                       name=f"psa{j}")
                        pss.append(psa)

                    def mk(k):
                        def slice_fn():
                            for j in range(NQ):
                                nc.tensor.matmul(
                                    pss[j][:],
                                    lhsT=xts[k][:],
                                    rhs=WA_sb[:, k * H + j * 512:
                                              k * H + (j + 1) * 512],
                                    start=(k == 0), stop=(k == DKT - 1),
                                )
                        return slice_fn

                    def finish():
                        ots = []
                        for j in range(NQ):
                            ot = otp.tile([128, 512], f16)
                            nc.vector.tensor_add(
                                ot[:], pss[j][:],
                                bias_sb[:, j * 512:(j + 1) * 512])
                            ots.append(ot)
                        return ots

                    return [mk(k) for k in range(DKT)], finish

                recurrence(0, W_sb, prefetchA, produceA)

            # ================= scope 2: phase C + rec1 ===================
            load_bias(bias_sb, b1)
            with (
                tc.tile_pool(name="wih1", bufs=1) as wap,
                tc.tile_pool(name="whh1", bufs=1) as wp,
                tc.tile_pool(name="lh", bufs=2) as lhp,
                tc.tile_pool(name="pc", bufs=1, space="PSUM") as pcp,
                tc.tile_pool(name="otc", bufs=12) as otp,
            ):
                WA_sb = wap.tile([128, KT * H], f16)      # 64KB/partition
                load_w(WA_sb, w_ih1, KT)
                W_sb = wp.tile([128, KT * H], f16)        # 64KB/partition
                load_w(W_sb, w_hh1, KT)

                lh_ring = {}

                def prefetchC(c):
                    lh0 = lhp.tile([128, 8 * 512], f16)
                    nc.gpsimd.dma_start(
                        lh0[:].rearrange("p (t c) -> p t c", t=8),
                        y0T_d[c * 8:(c + 1) * 8, :, :].rearrange("t p c -> p t c"),
                    )
                    lh_ring[c] = lh0

                def produceC(c):
                    lh0 = lh_ring.pop(c)
                    # y0T col t*512 + m*128 + 32j + b (b<16 valid) ->
                    # lh col (4j+m)*128 + t*16 + b so each k-tile's 128
                    # stationary columns are contiguous
                    lh = lhp.tile([128, 8 * KT * BL], f16)
                    nc.vector.tensor_copy(
                        lh[:].rearrange("p (j m2 t b) -> p j m2 t b",
                                        j=4, m2=4, t=8),
                        lh0[:].rearrange("p (t m2 j2 b2) -> p j2 m2 t b2",
                                         t=8, m2=4, j2=4)[:, :, :, :, 0:BL],
                    )
                    pss = []
                    for j in range(NQ):
                        psc = pcp.tile([128, 512], f32, space="PSUM",
                                       name=f"psc{j}")
                        pss.append(psc)

                    def mk(s):
                        def slice_fn():
                            for k in (2 * s, 2 * s + 1):
                                for j in range(NQ):
                                    nc.tensor.matmul(
                                        pss[j][:],
                                        lhsT=lh[:, k * 128:(k + 1) * 128],
                                        rhs=WA_sb[:, k * H + j * 512:
                                                  k * H + (j + 1) * 512],
                                        start=(k == 0), stop=(k == KT - 1),
                                    )
                        return slice_fn

                    def finish():
                        ots = []
                        for j in range(NQ):
                            ot = otp.tile([128, 512], f16)
                            nc.vector.tensor_add(
                                ot[:], pss[j][:],
                                bias_sb[:, j * 512:(j + 1) * 512])
                            ots.append(ot)
                        return ots

                    return [mk(s) for s in range(8)], finish

                recurrence(1, W_sb, prefetchC, produceC)

            # final capture: out[b] = h1 (fp16, already tanh'd) at t = len_b-1
            with tc.tile_pool(name="cap", bufs=1) as cp:
                ci = cp.tile([128, 1], i32)
                nc.gpsimd.dma_start(ci[:], cap_idx[:, :])
                og = cp.tile([128, 512], f16)
                nc.gpsimd.indirect_dma_start(
                    out=og[:], out_offset=None,
                    in_=h1s_d[:],
                    in_offset=bass.IndirectOffsetOnAxis(ap=ci[:, :1], axis=0),
                )
                oh = cp.tile([128, 512], f32)
                nc.vector.tensor_copy(oh[:], og[:])
                for j in range(NQ):
                    nc.gpsimd.dma_start(
                        out_h[:, j * 512:(j + 1) * 512],
                        oh[32 * j:32 * j + BL, :])

    nc.finalize()
    return nc


def _install_ntff_hook():
    """The trimmed agent image lacks antenv.axon_hooks — provide the tiny
    get/set registry and install the ctypes NTFF hook so trace=True works."""
    import types

    if "antenv.axon_hooks" in sys.modules:
        return
    m = types.ModuleType("antenv.axon_hooks")
    _hook = [None]
    m.set_axon_ntff_profile_hook = lambda h: _hook.__setitem__(0, h)
    m.get_axon_ntff_profile_hook = lambda: _hook[0]
    sys.modules["antenv.axon_hooks"] = m
    import antenv
    antenv.axon_hooks = m
    try:
        from trn_agent_boot.trn_boot import _ntff_profile_via_ctypes
        hook = _ntff_profile_via_ctypes("/opt/axon/libaxon_pjrt.so")
        if hook is not None:
            m.set_axon_ntff_profile_hook(hook)
        import concourse.bass_utils as bu
        bu.upload_artifacts = lambda d: str(d)
    except Exception:
        pass


def kernel(tokens, lengths, emb, W_ih0, W_hh0, b0, W_ih1, W_hh1, b1,
           _t_steps=T, _trace=False, _debug=False):
    from concourse.bass_utils import run_bass_kernel_spmd

    if _trace:
        _install_ntff_hook()

    tokens = np.asarray(tokens).astype(np.int32)
    lengths = np.asarray(lengths).astype(np.int32)
    emb16 = np.ascontiguousarray(np.asarray(emb, dtype=np.float16))
    W_ih0 = np.ascontiguousarray(np.asarray(W_ih0, dtype=np.float16))
    W_hh0 = np.ascontiguousarray(np.asarray(W_hh0, dtype=np.float16))
    W_ih1 = np.ascontiguousarray(np.asarray(W_ih1, dtype=np.float16))
    W_hh1 = np.ascontiguousarray(np.asarray(W_hh1, dtype=np.float16))
    b0 = np.ascontiguousarray(np.asarray(b0, dtype=np.float32).reshape(1, H))
    b1 = np.ascontiguousarray(np.asarray(b1, dtype=np.float32).reshape(1, H))

    ts = _t_steps
    key = (ts, _debug)
    if key not in _CACHE:
        _CACHE[key] = _build(ts, _debug)
    nc = _CACHE[key]

    in_maps = []
    for c in range(NC):
        tok_c = tokens[c * BL:(c + 1) * BL, :ts]          # [16, ts]
        flat = tok_c.T.reshape(-1)                        # t-major rows
        tokTc = np.ascontiguousarray(flat.reshape(-1, 128).T)  # [128, mt]
        len_c = np.minimum(lengths[c * BL:(c + 1) * BL].astype(np.int64), ts)
        # capture row for og row 32j+b: (len_b-1)*128 + 32j + b (hstack
        # layout); rows 32j+16..32j+31 are dummies pointing at row 0
        cap = np.zeros((128, 1), np.int32)
        for j in range(4):
            cap[32 * j:32 * j + BL, 0] = (len_c - 1) * 128 + 32 * j + np.arange(BL)
        in_maps.append({
            "tokT": tokTc,
            "cap_idx": np.ascontiguousarray(cap),
            "emb": emb16,
            "w_ih0": W_ih0, "w_hh0": W_hh0, "b0": b0,
            "w_ih1": W_ih1, "w_hh1": W_hh1, "b1": b1,
        })

    res = run_bass_kernel_spmd(nc, in_maps, list(range(NC)), trace=_trace)
    STATS["exec_time_ns"] = res.exec_time_ns
    STATS["mean_exec_time_ns"] = res.mean_exec_time_ns
    STATS["scope_times"] = res.per_core_scope_times
    if _debug:
        STATS["debug"] = res.results
    out = np.concatenate([res.results[c]["out_h"] for c in range(NC)], axis=0)
    return out.astype(np.float32)


# revision 6
# speedup vs baseline: 1.8649x; 1.1737x over previous
"""Trainium2 Bass kernel for a 2-layer masked (ragged) Elman RNN — v3.

v2 structure (all-fp16 datapath, quadrant-packed PSUM, one wide ACT, fp16 PE
transposes, capture-from-hstack) plus:
  - phase A (embed + xp0) is interleaved INTO the layer-0 recurrence: one
    A-chunk (8 timesteps of xp0) is produced per 8 recurrence steps, handing
    xp to the recurrence through an SBUF ot-tile ring (no DRAM bounce).
  - phase C (xp1) is likewise interleaved into the layer-1 recurrence.
  - bulk GEMMs run k-outer/j-inner so a stationary tile is reused by 4
    consecutive matmuls.
  - per-step xp injection reads a 32-row aligned window of the ot tile and
    uses an identity-slice selector as the stationary (odd steps start at
    partition 16, which engines cannot address directly).

Weight residency per scope: {W_ih0 32KB, W_hh0 64KB} then {W_ih1 64KB,
W_hh1 64KB} per partition — both fit alongside ~40KB of working tiles.
"""

import sys

sys.path.insert(0, "/opt/trn_rl_repo")

import numpy as np

B, T, V, D, H = 128, 512, 32000, 1024, 2048
NC = 8
BL = B // NC          # 16 sequences per core
KT = H // 128         # 16 k-tiles of the hidden dim
DKT = D // 128        # 8 k-tiles of the embedding dim
NQ = 4                # 4 psum quadrants / n-blocks of 512

STATS = {}
_CACHE = {}


def _build(t_steps, debug=False):
    import concourse.bass as bass
    import concourse.mybir as mybir
    import concourse.tile as tile
    from concourse import bacc
    from concourse.masks import make_identity

    f32 = mybir.dt.float32
    f16 = mybir.dt.float16
    i32 = mybir.dt.int32
    Tanh = mybir.ActivationFunctionType.Tanh

    mt = (t_steps * BL) // 128   # 128-row token tiles == 8-step chunks
    nchunk = t_steps // 8
    assert mt == nchunk

    nc = bacc.Bacc("TRN2", target_bir_lowering=False, debug=False, num_devices=NC)

    tokT = nc.dram_tensor("tokT", [128, mt], i32, kind="ExternalInput")
    cap_idx = nc.dram_tensor("cap_idx", [128, 1], i32, kind="ExternalInput")
    emb = nc.dram_tensor("emb", [V, D], f16, kind="ExternalInput")
    w_ih0 = nc.dram_tensor("w_ih0", [D, H], f16, kind="ExternalInput")
    w_hh0 = nc.dram_tensor("w_hh0", [H, H], f16, kind="ExternalInput")
    b0 = nc.dram_tensor("b0", [1, H], f32, kind="ExternalInput")
    w_ih1 = nc.dram_tensor("w_ih1", [H, H], f16, kind="ExternalInput")
    w_hh1 = nc.dram_tensor("w_hh1", [H, H], f16, kind="ExternalInput")
    b1 = nc.dram_tensor("b1", [1, H], f32, kind="ExternalInput")
    out_h = nc.dram_tensor("out_h", [BL, H], f32, kind="ExternalOutput")

    kd = dict(kind="ExternalOutput") if debug else {}
    y0T_d = nc.dram_tensor("y0T_d", [t_steps, 128, KT * BL], f16, **kd)
    # layer-1 tanh output in hstack layout: row t*128 + 32j + b = h[b, j*512:...]
    h1s_d = nc.dram_tensor("h1s_d", [t_steps * 128, 512], f16, **kd)

    def load_w(W_sb, wsrc, ktiles):
        for k in range(ktiles):
            nc.gpsimd.dma_start(
                W_sb[:, k * H:(k + 1) * H], wsrc[k * 128:(k + 1) * 128, :])

    def load_bias(bias_sb, bsrc):
        nc.gpsimd.dma_start(bias_sb[0:1, :], bsrc[0:1, :])
        nc.gpsimd.partition_broadcast(bias_sb[:], bias_sb[0:1, :])

    with tile.TileContext(nc) as tc:
        with tc.tile_pool(name="state", bufs=1) as st:
            ident = st.tile([128, 128], f16)
            make_identity(nc, ident[:])
            bias_sb = st.tile([128, H], f32)
            zero_sb = st.tile([128, KT * BL], f16)
            nc.gpsimd.memset(zero_sb[:], 0.0)
            tokens_sb = st.tile([128, mt], i32)
            nc.gpsimd.dma_start(tokens_sb[:], tokT[:, :])

            # ---- fused recurrence + bulk-xp producer --------------------
            # prefetch(c): issue the chunk-c input DMAs (2 chunks ahead)
            # produce(c): emit the chunk-c GEMM, returns [ot_j tiles] whose
            #             rows are t_loc*16+b covering steps 8c..8c+7
            def recurrence(layer, W_sb, prefetch, produce):
                with (
                    nc.named_scope(f"rec{layer}"),
                    tc.tile_pool(name=f"st{layer}", bufs=2) as stp,
                    tc.tile_pool(name=f"hs{layer}", bufs=2) as hsp,
                    tc.tile_pool(name=f"zt{layer}", bufs=3, space="PSUM") as ztp,
                    tc.tile_pool(name=f"tb{layer}", bufs=1, space="PSUM") as tbp,
                ):
                    ring = {}
                    prefetch(0)
                    if nchunk > 1:
                        prefetch(1)
                    ring[0] = produce(0)
                    hT_sb = stp.tile([128, KT * BL], f16, tag="hT")
                    nc.vector.tensor_copy(hT_sb[:], zero_sb[:])
                    for t in range(t_steps):
                        c, r = t // 8, t % 8
                        if r == 0:
                            if c + 2 < nchunk:
                                prefetch(c + 2)
                            if c + 1 < nchunk:
                                ring[c + 1] = produce(c + 1)
                            ring.pop(c - 1, None)
                        ots = ring[c]
                        w, half = (r // 2) * 32, r % 2
                        zt = ztp.tile([128, 512], f32, space="PSUM")
                        for j in range(NQ):
                            # zt[32j+b, :] = ot_j[r*16+b, :] via an identity
                            # selector on a 32-aligned window
                            nc.tensor.matmul(
                                zt[32 * j:32 * j + BL, :],
                                lhsT=ident[w:w + 32,
                                           w + half * BL:w + half * BL + BL],
                                rhs=ots[j][w:w + 32, :],
                                start=True, stop=False,
                                tile_position=(w, 32 * j),
                            )
                        for m in range(4):
                            for k in (m, m + 4, m + 8, m + 12):
                                for j in range(NQ):
                                    nc.tensor.matmul(
                                        zt[32 * j:32 * j + BL, :],
                                        lhsT=hT_sb[:, k * BL:(k + 1) * BL],
                                        rhs=W_sb[:, k * H + j * 512:
                                                 k * H + (j + 1) * 512],
                                        start=False, stop=(k == KT - 1),
                                        tile_position=(0, 32 * j),
                                    )
                        hs = hsp.tile([128, 512], f16)
                        hT_next = stp.tile([128, KT * BL], f16, tag="hT")
                        tb = tbp.tile([128, 512], f16, space="PSUM")
                        # ACT split per col-block so transpose m can start as
                        # soon as its block of tanh lands
                        for m in range(4):
                            nc.scalar.activation(
                                hs[:, m * 128:(m + 1) * 128],
                                zt[:, m * 128:(m + 1) * 128], Tanh)
                            nc.tensor.transpose(
                                tb[:, m * 128:(m + 1) * 128],
                                hs[:, m * 128:(m + 1) * 128],
                                ident[:],
                            )
                            nc.vector.tensor_copy(
                                hT_next[:].rearrange(
                                    "p (j m2 b) -> p m2 j b", j=4, m2=4)[:, m],
                                tb[:, m * 128:(m + 1) * 128]
                                .rearrange("p (j c) -> p j c", j=4)[:, :, 0:BL])
                        if layer == 1:
                            nc.gpsimd.dma_start(
                                h1s_d[t * 128:(t + 1) * 128, :], hs[:])
                        if layer == 0:
                            nc.gpsimd.dma_start(y0T_d[t, :, :], hT_next[:])
                        hT_sb = hT_next

            # ================= scope 1: phase A + rec0 ===================
            load_bias(bias_sb, b0)
            with (
                tc.tile_pool(name="wih0", bufs=1) as wap,
                tc.tile_pool(name="whh0", bufs=1) as wp,
                tc.tile_pool(name="ga", bufs=3) as gp,
                tc.tile_pool(name="xt", bufs=2) as xtp,
                tc.tile_pool(name="pa", bufs=1, space="PSUM") as pap,
                tc.tile_pool(name="ota", bufs=12) as otp,
            ):
                WA_sb = wap.tile([128, DKT * H], f16)     # 32KB/partition
                load_w(WA_sb, w_ih0, DKT)
                W_sb = wp.tile([128, KT * H], f16)        # 64KB/partition
                load_w(W_sb, w_hh0, KT)

                xg_ring = {}

                def prefetchA(c):
                    xg = gp.tile([128, D], f16)
                    nc.gpsimd.indirect_dma_start(
                        out=xg[:], out_offset=None,
                        in_=emb[:],
                        in_offset=bass.IndirectOffsetOnAxis(
                            ap=tokens_sb[:, c:c + 1], axis=0),
                    )
                    # XBAR-transpose each k-block now; latency hides behind
                    # the 2-chunk prefetch distance
                    xts = []
                    for k in range(DKT):
                        xtm = xtp.tile([128, 128], f16, name=f"xt{k}")
                        nc.sync.dma_start_transpose(
                            xtm[:], xg[:, k * 128:(k + 1) * 128])
                        xts.append(xtm)
                    xg_ring[c] = xts

                def produceA(c):
                    # xts were already transposed via XBAR at prefetch time
                    xts = xg_ring.pop(c)
                    pss = []
                    for j in range(NQ):
                        psa = pap.tile([128, 512], f32, space="PSUM",
                                       name=f"psa{j}")
                        pss.append(psa)
                    for k in range(DKT):
                        for j in range(NQ):
                            nc.tensor.matmul(
                                pss[j][:],
                                lhsT=xts[k][:],
                                rhs=WA_sb[:, k * H + j * 512:k * H + (j + 1) * 512],
                                start=(k == 0), stop=(k == DKT - 1),
                            )
                    ots = []
                    for j in range(NQ):
                        ot = otp.tile([128, 512], f16)
                        nc.vector.tensor_add(
                            ot[:], pss[j][:], bias_sb[:, j * 512:(j + 1) * 512])
                        ots.append(ot)
                    return ots

                recurrence(0, W_sb, prefetchA, produceA)

            # ================= scope 2: phase C + rec1 ===================
            load_bias(bias_sb, b1)
            with (
                tc.tile_pool(name="wih1", bufs=1) as wap,
                tc.tile_pool(name="whh1", bufs=1) as wp,
                tc.tile_pool(name="lh", bufs=2) as lhp,
                tc.tile_pool(name="pc", bufs=1, space="PSUM") as pcp,
                tc.tile_pool(name="otc", bufs=12) as otp,
            ):
                WA_sb = wap.tile([128, KT * H], f16)      # 64KB/partition
                load_w(WA_sb, w_ih1, KT)
                W_sb = wp.tile([128, KT * H], f16)        # 64KB/partition
                load_w(W_sb, w_hh1, KT)

                lh_ring = {}

                def prefetchC(c):
                    lh0 = lhp.tile([128, 8 * KT * BL], f16)
                    nc.gpsimd.dma_start(
                        lh0[:].rearrange("p (t c) -> p t c", t=8),
                        y0T_d[c * 8:(c + 1) * 8, :, :].rearrange("t p c -> p t c"),
                    )
                    lh_ring[c] = lh0

                def produceC(c):
                    lh0 = lh_ring.pop(c)
                    # (t,k,b) -> (k,t,b) so each k-tile's 128 stationary
                    # columns are contiguous
                    lh = lhp.tile([128, 8 * KT * BL], f16)
                    nc.vector.tensor_copy(
                        lh[:].rearrange("p (k t b) -> p k t b", k=KT, t=8),
                        lh0[:].rearrange("p (t k b) -> p k t b", t=8, k=KT),
                    )
                    pss = []
                    for j in range(NQ):
                        psc = pcp.tile([128, 512], f32, space="PSUM",
                                       name=f"psc{j}")
                        pss.append(psc)
                    for k in range(KT):
                        for j in range(NQ):
                            nc.tensor.matmul(
                                pss[j][:],
                                lhsT=lh[:, k * 128:(k + 1) * 128],
                                rhs=WA_sb[:, k * H + j * 512:k * H + (j + 1) * 512],
                                start=(k == 0), stop=(k == KT - 1),
                            )
                    ots = []
                    for j in range(NQ):
                        ot = otp.tile([128, 512], f16)
                        nc.vector.tensor_add(
                            ot[:], pss[j][:], bias_sb[:, j * 512:(j + 1) * 512])
                        ots.append(ot)
                    return ots

                recurrence(1, W_sb, prefetchC, produceC)

            # final capture: out[b] = h1 (fp16, already tanh'd) at t = len_b-1
            with tc.tile_pool(name="cap", bufs=1) as cp:
                ci = cp.tile([128, 1], i32)
                nc.gpsimd.dma_start(ci[:], cap_idx[:, :])
                og = cp.tile([128, 512], f16)
                nc.gpsimd.indirect_dma_start(
                    out=og[:], out_offset=None,
                    in_=h1s_d[:],
                    in_offset=bass.IndirectOffsetOnAxis(ap=ci[:, :1], axis=0),
                )
                oh = cp.tile([128, 512], f32)
                nc.vector.tensor_copy(oh[:], og[:])
                for j in range(NQ):
                    nc.gpsimd.dma_start(
                        out_h[:, j * 512:(j + 1) * 512],
                        oh[32 * j:32 * j + BL, :])

    nc.finalize()
    return nc


def _install_ntff_hook():
    """The trimmed agent image lacks antenv.axon_hooks — provide the tiny
    get/set registry and install the ctypes NTFF hook so trace=True works."""
    import types

    if "antenv.axon_hooks" in sys.modules:
        return
    m = types.ModuleType("antenv.axon_hooks")
    _hook = [None]
    m.set_axon_ntff_profile_hook = lambda h: _hook.__setitem__(0, h)
    m.get_axon_ntff_profile_hook = lambda: _hook[0]
    sys.modules["antenv.axon_hooks"] = m
    import antenv
    antenv.axon_hooks = m
    try:
        from trn_agent_boot.trn_boot import _ntff_profile_via_ctypes
        hook = _ntff_profile_via_ctypes("/opt/axon/libaxon_pjrt.so")
        if hook is not None:
            m.set_axon_ntff_profile_hook(hook)
        import concourse.bass_utils as bu
        bu.upload_artifacts = lambda d: str(d)
    except Exception:
        pass


def kernel(tokens, lengths, emb, W_ih0, W_hh0, b0, W_ih1, W_hh1, b1,
           _t_steps=T, _trace=False, _debug=False):
    from concourse.bass_utils import run_bass_kernel_spmd

    if _trace:
        _install_ntff_hook()

    tokens = np.asarray(tokens).astype(np.int32)
    lengths = np.asarray(lengths).astype(np.int32)
    emb16 = np.ascontiguousarray(np.asarray(emb, dtype=np.float16))
    W_ih0 = np.ascontiguousarray(np.asarray(W_ih0, dtype=np.float16))
    W_hh0 = np.ascontiguousarray(np.asarray(W_hh0, dtype=np.float16))
    W_ih1 = np.ascontiguousarray(np.asarray(W_ih1, dtype=np.float16))
    W_hh1 = np.ascontiguousarray(np.asarray(W_hh1, dtype=np.float16))
    b0 = np.ascontiguousarray(np.asarray(b0, dtype=np.float32).reshape(1, H))
    b1 = np.ascontiguousarray(np.asarray(b1, dtype=np.float32).reshape(1, H))

    ts = _t_steps
    key = (ts, _debug)
    if key not in _CACHE:
        _CACHE[key] = _build(ts, _debug)
    nc = _CACHE[key]

    in_maps = []
    for c in range(NC):
        tok_c = tokens[c * BL:(c + 1) * BL, :ts]          # [16, ts]
        flat = tok_c.T.reshape(-1)                        # t-major rows
        tokTc = np.ascontiguousarray(flat.reshape(-1, 128).T)  # [128, mt]
        len_c = np.minimum(lengths[c * BL:(c + 1) * BL].astype(np.int64), ts)
        # capture row for og row 32j+b: (len_b-1)*128 + 32j + b (hstack
        # layout); rows 32j+16..32j+31 are dummies pointing at row 0
        cap = np.zeros((128, 1), np.int32)
        for j in range(4):
            cap[32 * j:32 * j + BL, 0] = (len_c - 1) * 128 + 32 * j + np.arange(BL)
        in_maps.append({
            "tokT": tokTc,
            "cap_idx": np.ascontiguousarray(cap),
            "emb": emb16,
            "w_ih0": W_ih0, "w_hh0": W_hh0, "b0": b0,
            "w_ih1": W_ih1, "w_hh1": W_hh1, "b1": b1,
        })

    res = run_bass_kernel_spmd(nc, in_maps, list(range(NC)), trace=_trace)
    STATS["exec_time_ns"] = res.exec_time_ns
    STATS["mean_exec_time_ns"] = res.mean_exec_time_ns
    STATS["scope_times"] = res.per_core_scope_times
    if _debug:
        STATS["debug"] = res.results
    out = np.concatenate([res.results[c]["out_h"] for c in range(NC)], axis=0)
    return out.astype(np.float32)
